# revision 1
# baseline (speedup 1.0000x reference)
"""Multistep LIF forward (T=4) on 8 Trainium2 NeuronCores.

Data-parallel over the batch dim (32 -> 4 per core). Each core streams its
shard through SBUF in [128, FREE] tiles; the T-step scan state stays in SBUF.

Raw Bass (no Tile): the walrus codegen in this toolchain encodes at most ONE
sync-wait per HW instruction, so all cross-engine waits are standalone
wait_ge instructions and every data instruction carries only sem updates.

Engine split per step g=(chunk c, time t):
  DVE   : u = v + x_t ; m = (u<=1) ; mem = u*m          (scan chain)
  ACT   : s = 1-m ; v' = 0.5*mem ; issues the two stores (qActDynamicHW)
  SP    : issues x loads (qSPDynamicHW)
Per-SBUF-slot DMA semaphores make completion tracking order-independent.
"""

import sys
from contextlib import ExitStack

import numpy as np

for _p in ("/opt/trn_rl_repo",):
    if _p not in sys.path:
        sys.path.insert(0, _p)

T, B, H, W = 4, 32, 512, 1024
NCORES = 8
BS = B // NCORES            # batch rows per core
PART = 128
FREE = 4096
CH = (BS * H * W) // (PART * FREE)   # chunks per timestep per core
VTHR = 1.0
TAU = 0.5
NXB = 2                     # x-tile ring depth
NOB = 2                     # output-tile ring depth

_NC = None


def _build_nc(ch=CH, free=FREE):
    import concourse.bass as bass
    from concourse import mybir

    f32 = mybir.dt.float32
    alu = mybir.AluOpType
    AF = mybir.ActivationFunctionType

    nc = bass.Bass()
    x_d = nc.declare_dram_parameter("x", [T, ch, PART, free], f32, isOutput=False)
    s_d = nc.declare_dram_parameter("spikes", [T, ch, PART, free], f32, isOutput=True)
    m_d = nc.declare_dram_parameter("mems", [T, ch, PART, free], f32, isOutput=True)

    # cumulative cp_sem increments once step g has fully retired on DVE:
    # t==0 contributes 2 (m, mem); t>0 contributes 3 (add, m, mem)
    def cpa(g):
        return 11 * (g // T) + (0, 2, 5, 8, 11)[g % T + 1]

    with ExitStack() as ctx:
        xt = [ctx.enter_context(nc.sbuf_tensor(f"xt{i}", [PART, free], f32)) for i in range(NXB)]
        st = [ctx.enter_context(nc.sbuf_tensor(f"st{i}", [PART, free], f32)) for i in range(NOB)]
        mt = [ctx.enter_context(nc.sbuf_tensor(f"mt{i}", [PART, free], f32)) for i in range(NOB)]
        u_s = ctx.enter_context(nc.sbuf_tensor("u_s", [PART, free], f32))
        v_s = ctx.enter_context(nc.sbuf_tensor("v_s", [PART, free], f32))
        m_s = [ctx.enter_context(nc.sbuf_tensor(f"m_s{i}", [PART, free], f32)) for i in range(2)]
        xsem = [ctx.enter_context(nc.semaphore(f"xsem{i}")) for i in range(NXB)]
        sts = [ctx.enter_context(nc.semaphore(f"sts{i}")) for i in range(NOB)]
        stm = [ctx.enter_context(nc.semaphore(f"stm{i}")) for i in range(NOB)]
        cp_sem = ctx.enter_context(nc.semaphore("cp_sem"))
        act_sem = ctx.enter_context(nc.semaphore("act_sem"))
        block = ctx.enter_context(nc.Block())

        def s_store(sync, g):
            # spike store for step g on the SP ring (balances the two HWDGE
            # rings: loads + s-stores here, mem-stores on the ACT ring)
            c, t = divmod(g, T)
            ob = g % NOB
            sync.wait_ge(act_sem, 2 * g + 1)
            sync.dma_start(out=s_d[t, c], in_=st[ob][:]).then_inc(sts[ob], 16)

        @block.sync
        def _(sync):
            for c in range(ch):
                for t in range(T):
                    g = c * T + t
                    b = g % NXB
                    if g >= NXB:
                        # slot's previous x fully consumed by DVE step g-NXB
                        sync.wait_ge(cp_sem, cpa(g - NXB))
                    sync.dma_start(out=xt[b][:], in_=x_d[t, c]).then_inc(xsem[b], 16)
                    if g >= 1:
                        s_store(sync, g - 1)
            s_store(sync, ch * T - 1)

        @block.vector
        def _(vector):
            cp = 0
            for c in range(ch):
                for t in range(T):
                    g = c * T + t
                    b = g % NXB
                    ob = g % NOB
                    mb = g % 2
                    vector.wait_ge(xsem[b], 16 * (g // NXB + 1))
                    if g >= 1:
                        # ACT through step g-1 done: v' ready (t>0) and the
                        # old m_s[mb] reader (s of step g-2) finished
                        vector.wait_ge(act_sem, 2 * g)
                        # same-engine WAR/RAW catch-all for prior steps
                        vector.wait_ge(cp_sem, cpa(g - 1))
                    if t == 0:
                        u = xt[b]
                    else:
                        u = u_s
                        nc.vector.tensor_tensor(
                            u[:], v_s[:], xt[b][:], op=alu.add
                        ).then_inc(cp_sem, 1)
                        cp += 1
                        vector.wait_ge(cp_sem, cp)  # engine pipeline drain
                    nc.vector.tensor_scalar(
                        m_s[mb][:], u[:], VTHR, None, op0=alu.is_le
                    ).then_inc(cp_sem, 1)
                    cp += 1
                    vector.wait_ge(cp_sem, cp)
                    if g >= NOB:
                        # previous store from this mem slot drained
                        vector.wait_ge(stm[ob], 16 * (g // NOB))
                    nc.vector.tensor_tensor(
                        mt[ob][:], u[:], m_s[mb][:], op=alu.mult
                    ).then_inc(cp_sem, 1)
                    cp += 1

        @block.scalar
        def _(scalar):
            for c in range(ch):
                for t in range(T):
                    g = c * T + t
                    ob = g % NOB
                    mb = g % 2
                    scalar.wait_ge(cp_sem, cpa(g))
                    if g >= NOB:
                        scalar.wait_ge(sts[ob], 16 * (g // NOB))
                    nc.scalar.activation(
                        st[ob][:], m_s[mb][:], AF.Copy, bias=1.0, scale=-1.0
                    ).then_inc(act_sem, 1)
                    # decay for the carried state (computed every step for a
                    # uniform act_sem count; t=3's result is unused)
                    nc.scalar.activation(
                        v_s[:], mt[ob][:], AF.Copy, bias=0.0, scale=TAU
                    ).then_inc(act_sem, 1)
                    scalar.wait_ge(act_sem, 2 * g + 2)
                    scalar.dma_start(out=m_d[t, c], in_=mt[ob][:]).then_inc(stm[ob], 16)

    return nc


def _get_nc():
    global _NC
    if _NC is None:
        _NC = _build_nc()
    return _NC


def _run(x_np, trace=False, **spmd_kwargs):
    from concourse.bass_utils import run_bass_kernel_spmd

    nc = _get_nc()
    in_maps = []
    for k in range(NCORES):
        shard = np.ascontiguousarray(
            x_np[:, k * BS:(k + 1) * BS].reshape(T, CH, PART, FREE)
        )
        in_maps.append({"x": shard})
    res = run_bass_kernel_spmd(
        nc, in_maps, list(range(NCORES)), trace=trace, **spmd_kwargs
    )
    spikes = np.empty((T, B, H, W), dtype=np.float32)
    mems = np.empty((T, B, H, W), dtype=np.float32)
    for k in range(NCORES):
        spikes[:, k * BS:(k + 1) * BS] = np.asarray(
            res.results[k]["spikes"]
        ).reshape(T, BS, H, W)
        mems[:, k * BS:(k + 1) * BS] = np.asarray(
            res.results[k]["mems"]
        ).reshape(T, BS, H, W)
    return (spikes, mems), res


def kernel(x, **_ignored):
    x_np = np.asarray(x, dtype=np.float32)
    return _run(x_np)[0]

